# revision 69
# baseline (speedup 1.0000x reference)
"""Trainium2 Bass kernel for nn_ChebConvNet (ChebConv K=1 => 3-layer MLP + log_softmax).

Computation per node row (edge_index is inert for K=1 ChebConv):
    h = silu(x @ W0 + b0); h = silu(h @ W1 + b1); h2 = h @ W2 + b2
    out = log_softmax(h2, axis=1)

Strategy: shard the 500k node rows across 8 NeuronCores (row-parallel, no
communication). ACT is the binding engine (~160us of SiLU+exp+ln at 128
lanes); everything else is built to keep it streaming:
  - x is staged in DRAM as bf16 (cast on host, same rounding the previous
    device-side cast-DMA produced) and loaded FEATURE-MAJOR via the HWDGE
    XBAR transpose path, so no PE transposes / DVE copies / PSUM staging of
    x are needed at all. Loads run 3 pairs ahead on SP's queue.
  - per 1024-row macro: mm1 (2x512 chunks) -> fp32 PSUM (double buffered),
    SiLU0 -> bf16 SBUF (bias via the ACT bias operand), mm2, SiLU1, then mm3
    with h1 row-PAIRED as the stationary operand so h2 comes out row-major
    and the final store uses 512-byte descriptors.
  - all five weight/bias tensors ride ONE bf16 XBAR-transposed DMA (host
    stages b0|b1|w0.T|w1.T|w2.T rows), keeping the prologue pure-transpose
    on the DMA queue and landing each weight pre-oriented for its matmul.
  - softmax tail: h2 parks as bf16 (DVE), exp on ACT in 4096-wide chunks
    over 8-macro groups, the group ln deferred 3 iterations so the DVE
    reduce is never on ACT's critical path, subtract on GPSIMD (otherwise
    idle), row-paired stores per 2048 rows. The last ~13 macros use
    2-macro groups (short ln defer) so their subtracts/stores overlap the
    remaining SiLU stream instead of bunching into a serial store chain;
    the final macro is a singleton group whose softmax runs straight out
    of PSUM (on DVE - GPSIMD cannot access PSUM).
  - per-core rows = ceil(n/8) rounded to 16 (62512 for n=500k): 61 full
    macros plus one 48-row tail unit (its own mm1/silu/mm2/silu/mm3 and
    PSUM-direct softmax on <=128 partitions, row-major store), instead of
    padding every core to 62 macros - ACT, PE and DMA all skip the ~1.5%
    of padded work the old 2048-row granularity forced.
"""

import math
import numpy as np
import ml_dtypes

import bass_rust
import concourse.bass as bass
import concourse.tile as tile
from concourse import mybir
from concourse.bass_utils import run_bass_kernel_spmd
from concourse.vector_clock import ScopedClock
from bass_rust import add_dep_helper

N_CORES = 8
F_IN = 128
F_HID = 128
F_OUT = 64
MACRO_ROWS = 1024      # rows per macro (PSUM-limited unit)
PAIR_ROWS = 2048       # rows per DMA load / store / silu0 unit
BATCH_ROWS = PAIR_ROWS  # row-count granularity for per-core padding
MACRO_FREE = (MACRO_ROWS // 128) * F_OUT   # h2 free elems per macro (512)
MACRO_BLKS = MACRO_ROWS // 128             # 64-wide row blocks per macro (8)
GROUP_MACROS = 8       # macros per exp/ln group (exp chunk = 4096 free)
CPACK_ROWS = 336       # b0|b1|pad14|w0.T(128)|w1.T(128)|w2.T(64) rows

_DT = mybir.dt

# this walrus build rejects instructions with more than ONE sync wait; the
# Tile framework freely assigns several. Two patches below: (1) split every
# multi-wait instruction by inserting single-wait NoOp carriers on the same
# engine right before it (order on the engine's sequencer preserves
# semantics); (2) the TileContext tail drain gets the same treatment with
# single-wait drain carriers.
_MAX_DRAIN_WAITS = 1
_N_SPARE_DRAINS = 20

_NOOP_CLS = None
_carrier_counter = [0]


def _noop_cls():
    global _NOOP_CLS
    if _NOOP_CLS is None:
        _NOOP_CLS = getattr(bass_rust, "InstNoOp")
    return _NOOP_CLS


_orig_lower_ordered = tile.TileContext._lower_ordered_insts


def _split_multi_waits(self, ordered):
    cls = _noop_cls()
    new_ordered = {}
    for bb_name, insts in ordered.items():
        new_list = []
        for inst in insts:
            si = inst.sync_info
            waits = list(si.on_wait) if si is not None else []
            if len(waits) > 1:
                for w in waits[:-1]:
                    c = cls(name=f"waitcar-{_carrier_counter[0]}", ins=[],
                            outs=[])
                    _carrier_counter[0] += 1
                    c.engine = inst.engine
                    c.sync_info = bass_rust.SyncInfo(on_wait=[w], on_update=[])
                    new_list.append(c)
                inst.sync_info = bass_rust.SyncInfo(
                    on_wait=[waits[-1]], on_update=list(si.on_update))
            new_list.append(inst)
        new_ordered[bb_name] = new_list
    return _orig_lower_ordered(self, new_ordered)


tile.TileContext._lower_ordered_insts = _split_multi_waits


def _patched_drain_and_barrier(self, tick_clock, wait_clock):
    nc = self.nc
    spare = [nc.sync.drain() for _ in range(_N_SPARE_DRAINS)]
    drain_inst = nc.sync.drain()
    wait_clock.add_sem_waits(
        drain_inst.ins, ScopedClock({None: tick_clock.global_clock})
    )
    si = drain_inst.ins.sync_info
    waits = list(si.on_wait) if si is not None else []
    if len(waits) > _MAX_DRAIN_WAITS:
        chunks = [
            waits[i : i + _MAX_DRAIN_WAITS]
            for i in range(0, len(waits), _MAX_DRAIN_WAITS)
        ]
        head, tail = chunks[:-1], chunks[-1]
        assert len(head) <= _N_SPARE_DRAINS, "bump _N_SPARE_DRAINS"
        for nop_i, chunk in zip(spare, head):
            nop_i.ins.sync_info = bass_rust.SyncInfo(on_wait=chunk, on_update=[])
        drain_inst.ins.sync_info = bass_rust.SyncInfo(
            on_wait=tail, on_update=list(si.on_update)
        )
    nc.all_engine_barrier()
    assert self.sems is not None
    popped = nc._tile_sem_poison_stack.pop()
    assert popped is self._sem_poison
    nc.clear_and_free_semaphores(list(self.sems.allocated().values()))
    nc.all_engine_barrier()


tile.TileContext._drain_and_barrier = _patched_drain_and_barrier


def _make_groups(n_macros, max_singles=2):
    """Exp/ln group sizes: full-size groups with a tapered (smaller, even)
    tail so the final un-overlapped softmax segment is short. At most
    `max_singles` trailing singleton groups (each keeps an h2 PSUM tile
    alive through its subtract; the 48-row tail unit needs one slot too)."""
    groups = []
    rem = n_macros
    while rem > 14:
        groups.append(GROUP_MACROS)
        rem -= GROUP_MACROS
    # the last ~14 macros use 2-macro groups: their lns (short defer) land
    # while ACT still streams silus, so the tail subtracts/stores overlap
    # instead of bunching into a serial store chain at the very end
    while rem > max_singles:
        groups.append(2)
        rem -= 2
    while rem > 0:
        groups.append(1)
        rem -= 1
    return groups


def _build(nc_rows: int, with_b2: bool):
    """Build the per-core Bass module. nc_rows = k*1024 + tail with
    tail in {0} or [16..128] (16-aligned): full macros plus one small
    tail unit, so the padded row count tracks ceil(n/8) closely."""
    tail_rows = nc_rows % MACRO_ROWS
    assert tail_rows == 0 or (16 <= tail_rows <= 128 and tail_rows % 16 == 0)
    nc = bass.Bass("TRN2", target_bir_lowering=False, debug=False,
                   num_devices=N_CORES)

    x_d = nc.dram_tensor("x", [nc_rows, F_IN], _DT.bfloat16,
                         kind="ExternalInput").ap()
    # all small weights/biases ride ONE XBAR-transposed dma, the same DMA
    # type as the x loads (transpose<->copy mode switches on the HWDGE queue
    # serialize on a full completion wait, so the prologue stays
    # pure-transpose). Host stages rows b0 | b1 | 14 pad | w0.T | w1.T |
    # w2.T in bf16; the transposed load lands each weight in exactly the
    # orientation its matmul consumes.
    cpack_d = nc.dram_tensor("cpack", [CPACK_ROWS, F_IN], _DT.bfloat16,
                             kind="ExternalInput").ap()
    b2_d = nc.dram_tensor("b2", [1, F_OUT], _DT.bfloat16,
                          kind="ExternalInput").ap()
    out_d = nc.dram_tensor("out", [nc_rows, F_OUT], _DT.float32,
                           kind="ExternalOutput").ap()

    n_macros = nc_rows // MACRO_ROWS
    n_pairs = (n_macros + 1) // 2
    groups = _make_groups(n_macros, max_singles=1 if tail_rows else 2)
    # macro -> (group idx, offset in group); group -> (start, size)
    g_start = []
    acc = 0
    for sz in groups:
        g_start.append(acc)
        acc += sz
    assert acc == n_macros
    macro_group = {}
    for gi, (st, sz) in enumerate(zip(g_start, groups)):
        for off in range(sz):
            macro_group[st + off] = (gi, off)

    AF = mybir.ActivationFunctionType

    with tile.TileContext(nc) as tc:
        with (
            tc.tile_pool(name="consts", bufs=1) as consts,
            tc.tile_pool(name="xt", bufs=4) as xts,
            tc.tile_pool(name="h0_ps", bufs=2, space="PSUM") as h0p,
            tc.tile_pool(name="h1_ps", bufs=1, space="PSUM") as h1p,
            tc.tile_pool(name="h2_ps", bufs=2, space="PSUM") as h2p,
            tc.tile_pool(name="h0_sb", bufs=4) as h0s,
            tc.tile_pool(name="h1_sb", bufs=4) as h1s,
            tc.tile_pool(name="park", bufs=6) as parkp,
            tc.tile_pool(name="e", bufs=4) as epool,
            tc.tile_pool(name="s", bufs=8) as spool,
            tc.tile_pool(name="lz", bufs=8) as lzpool,
            tc.tile_pool(name="o", bufs=6) as opool,
        ):

            xt_tiles = {}       # pair -> xt tile (feature-major x)
            h0b_tiles = {}      # macro -> h0b tile
            h1b_tiles = {}      # macro -> h1b tile
            h2_tiles = {}       # macro -> h2 PSUM tile
            park_tiles = {}     # group -> park tile
            s_tiles = {}
            lz_tiles = {}
            o_tiles = {}        # pair -> output tile
            h0t_tiles = {}      # macro -> h0 PSUM tile

            def load(p, split=1):
                xt = xts.tile([128, PAIR_ROWS], _DT.bfloat16, name="xt", tag="xt")
                r0 = p * PAIR_ROWS
                rows = min(PAIR_ROWS, nc_rows - r0)
                if rows < PAIR_ROWS:
                    i0 = nc.sync.dma_start(xt[:, 0:rows], x_d[r0:r0 + rows, :],
                                           transpose=True)
                    xt_tiles[p] = xt
                    return [i0]
                step = PAIR_ROWS // split
                insts = []
                for i in range(split):
                    insts.append(nc.sync.dma_start(
                        xt[:, i * step:(i + 1) * step],
                        x_d[r0 + i * step:r0 + (i + 1) * step, :],
                        transpose=True))
                xt_tiles[p] = xt
                return insts

            def mm1(m):
                p = m // 2
                half = m % 2
                h0t = h0p.tile([128, MACRO_ROWS], _DT.float32,
                               name="h0t", tag="h0t")
                h0t_tiles[m] = h0t
                xt = xt_tiles[p]
                for c in range(2):
                    nc.tensor.matmul(
                        h0t[:, c * 512:(c + 1) * 512],
                        lhsT=w0,
                        rhs=xt[:, half * 1024 + c * 512:half * 1024 + (c + 1) * 512],
                        start=True, stop=True)

            def silu0(m):
                h0b = h0s.tile([128, MACRO_ROWS], _DT.bfloat16, tag="h0b")
                nc.scalar.activation(h0b[:], h0t_tiles.pop(m)[:], AF.Silu,
                                     bias=b0)
                h0b_tiles[m] = h0b

            def mm2(m):
                h1t = h1p.tile([128, MACRO_ROWS], _DT.float32, tag="h1t")
                h0b = h0b_tiles.pop(m)
                for c in range(2):
                    nc.tensor.matmul(
                        h1t[:, c * 512:(c + 1) * 512], lhsT=w1,
                        rhs=h0b[:, c * 512:(c + 1) * 512],
                        start=True, stop=True)
                return h1t

            def silu1(m, h1t):
                h1b = h1s.tile([128, MACRO_ROWS], _DT.bfloat16, tag="h1b")
                nc.scalar.activation(h1b[:], h1t[:], AF.Silu, bias=b1)
                h1b_tiles[m] = h1b

            def mm3(m):
                # row-PAIRING: block b = (j, c2, s) covers rows
                # {512j + 256*c2 + 2q + s : q in 0..127}; adjacent s-blocks
                # make each partition's two rows CONSECUTIVE in DRAM, so the
                # store uses 512-byte descriptors instead of 256-byte ones.
                h2t = h2p.tile([128, MACRO_FREE], _DT.float32, tag="h2t")
                h1b = h1b_tiles.pop(m)
                n_mm3 = 8 * (2 if with_b2 else 1)
                k = 0
                for j in range(2):
                    for c2 in range(2):
                        for s in range(2):
                            b = j * 4 + c2 * 2 + s
                            lview = (h1b[:, j * 512 + c2 * 256:
                                          j * 512 + (c2 + 1) * 256]
                                     .rearrange("p (q two) -> p q two", two=2)
                                     [:, :, s])
                            nc.tensor.matmul(
                                h2t[:, b * 64:(b + 1) * 64],
                                lhsT=lview, rhs=w2,
                                start=(k == 0), stop=(k == n_mm3 - 1))
                            k += 1
                if with_b2:
                    for b in range(8):
                        nc.tensor.matmul(
                            h2t[:, b * 64:(b + 1) * 64],
                            lhsT=ones1[:], rhs=b2[:],
                            start=False, stop=(k == n_mm3 - 1))
                        k += 1
                h2_tiles[m] = h2t

            def parkcopy(m):
                gi, off = macro_group[m]
                if groups[gi] == 1:
                    # epilogue singleton group: softmax runs straight out of
                    # the PSUM tile; no park copy
                    park_tiles[gi] = h2_tiles.pop(m)
                    return
                if off == 0:
                    park_tiles[gi] = parkp.tile(
                        [128, GROUP_MACROS * MACRO_FREE], _DT.bfloat16,
                        name="park", tag="park")
                pk = park_tiles[gi]
                last_parkcopy[0] = nc.vector.tensor_copy(
                    pk[:, off * MACRO_FREE:(off + 1) * MACRO_FREE],
                    h2_tiles.pop(m)[:])

            def exp_reduce(gi):
                """exp for the whole group; the reduce is split in two and
                the second half is deferred an iteration (emit_reduce2) so
                the 4.3us reduce never blocks DVE's parkcopy chain."""
                sz = groups[gi]
                w = sz * MACRO_FREE
                pk = park_tiles[gi]
                e = epool.tile([128, GROUP_MACROS * MACRO_FREE], _DT.bfloat16,
                               tag="e")
                nc.scalar.activation(e[:, :w], pk[:, :w], AF.Exp)
                S = spool.tile([128, GROUP_MACROS * MACRO_BLKS], _DT.float32,
                               tag="s")
                half_m = (sz + 1) // 2
                wh = half_m * MACRO_FREE
                nc.vector.tensor_reduce(
                    S[:, :half_m * MACRO_BLKS],
                    e[:, :wh].rearrange("p (b f) -> p b f", f=F_OUT),
                    axis=mybir.AxisListType.X, op=mybir.AluOpType.add)
                s_tiles[gi] = S
                if sz > half_m:
                    pending_reduce2.append((gi, e, half_m))

            def emit_reduce2():
                gi, e, half_m = pending_reduce2.pop(0)
                sz = groups[gi]
                S = s_tiles[gi]
                r2 = nc.vector.tensor_reduce(
                    S[:, half_m * MACRO_BLKS:sz * MACRO_BLKS],
                    e[:, half_m * MACRO_FREE:sz * MACRO_FREE].rearrange(
                        "p (b f) -> p b f", f=F_OUT),
                    axis=mybir.AxisListType.X, op=mybir.AluOpType.add)
                if last_parkcopy[0] is not None:
                    # ordering-only edge: the bulky reduce must not delay the
                    # parkcopy chain (h2 PSUM recycling gates PE's mm3)
                    add_dep_helper(r2.ins, last_parkcopy[0].ins, sync=False,
                                   reason="parkcopy before reduce2")

            def ln_group(gi):
                sz = groups[gi]
                LZ = lzpool.tile([128, GROUP_MACROS * MACRO_BLKS],
                                 _DT.float32, tag="lz")
                nc.scalar.activation(LZ[:, :sz * MACRO_BLKS],
                                     s_tiles.pop(gi)[:, :sz * MACRO_BLKS],
                                     AF.Ln)
                lz_tiles[gi] = LZ

            def sub(m, engine):
                gi, off = macro_group[m]
                p = m // 2
                half = m % 2
                if p not in o_tiles:
                    o_tiles[p] = opool.tile([128, PAIR_ROWS // 128 * F_OUT],
                                            _DT.float32, name="o", tag="o")
                o = o_tiles[p]
                pk = park_tiles[gi]
                lzb = (lz_tiles[gi][:, off * MACRO_BLKS:(off + 1) * MACRO_BLKS]
                       .broadcast_to([128, MACRO_BLKS, F_OUT]))
                engine.tensor_tensor(
                    out=o[:, half * MACRO_FREE:(half + 1) * MACRO_FREE]
                    .rearrange("p (b f) -> p b f", f=F_OUT),
                    in0=pk[:, off * MACRO_FREE:(off + 1) * MACRO_FREE]
                    .rearrange("p (b f) -> p b f", f=F_OUT),
                    in1=lzb, op=mybir.AluOpType.subtract)

            def store(p, halves=1):
                # paired-row layout: block index (P, s) holds rows
                # base + 256*P + 2*q + s; (s, f) is 512B-contiguous in DRAM.
                # halves=2 splits into per-macro stores (epilogue: the first
                # half leaves while the last subtract still runs)
                o = o_tiles.pop(p)
                rows = PAIR_ROWS // halves
                for h in range(halves):
                    r0 = p * PAIR_ROWS + h * rows
                    nc.sync.dma_start(
                        out_d[r0:r0 + rows, :].rearrange(
                            "(P q s) f -> q P s f", q=128, s=2),
                        o[:, h * (rows // 128) * F_OUT:
                          (h + 1) * (rows // 128) * F_OUT].rearrange(
                            "p (P s f) -> p P s f", s=2, f=F_OUT))

            # ---- prologue: every prologue DMA is an XBAR transpose on
            # SP's queue (no mode switches, no cross-DGE-type waits); the
            # tiny consts transfer goes first so w0/b0 land before x0a.
            cw = consts.tile([128, CPACK_ROWS], _DT.bfloat16, tag="cw")
            nc.sync.dma_start(cw[:], cpack_d[:, :], transpose=True)
            load(0, split=2)
            w0 = cw[:, 16:144]
            w1 = cw[:, 144:272]
            w2 = cw[:, 272:336]
            b0t = consts.tile([128, 1], _DT.float32, tag="b0t")
            nc.vector.tensor_copy(b0t[:], cw[:, 0:1])
            b1t = consts.tile([128, 1], _DT.float32, tag="b1t")
            nc.vector.tensor_copy(b1t[:], cw[:, 1:2])
            b0 = b0t[:]
            b1 = b1t[:]
            b2 = None
            ones1 = None
            if with_b2:
                b2 = consts.tile([1, F_OUT], _DT.bfloat16, tag="b2")
                nc.scalar.dma_start(b2[:], b2_d[:, :])
                ones1 = consts.tile([1, 128], _DT.bfloat16, tag="ones1")
                nc.gpsimd.memset(ones1[:], 1.0)
            if n_pairs > 1:
                load(1)
            if n_pairs > 2:
                load(2)

            pending_subs = []   # macros whose ln is emitted, sub not yet
            pending_ln = []     # (group, ready_iter) exp/reduce emitted
            pending_reduce2 = []  # deferred second reduce halves
            last_parkcopy = [None]
            sub_done = {}       # pair -> count of subs emitted

            def sub_engine(m2):
                # steady state on Pool (DVE carries parkcopy+reduce); in the
                # tail groups alternate DVE/Pool so the final subtracts run
                # in parallel instead of serializing on Pool. Singleton
                # groups read h2 straight from PSUM, which GPSIMD cannot
                # touch - those must run on DVE.
                gi2 = macro_group[m2][0]
                if groups[gi2] == 1:
                    return nc.vector
                if gi2 >= len(groups) - 3:
                    return nc.vector if m2 % 2 == 1 else nc.gpsimd
                return nc.gpsimd

            last_pair = n_pairs - 1

            def finish_sub(m2):
                p2 = m2 // 2
                sub_done[p2] = sub_done.get(p2, 0) + 1
                if p2 == last_pair:
                    store_half(p2, m2 % 2)
                elif p2 == last_pair - 1 and n_macros % 2 == 1:
                    store_half(p2, m2 % 2)
                elif sub_done[p2] == 2:
                    store(p2)

            def store_half(p, h):
                rows = MACRO_ROWS
                r0 = p * PAIR_ROWS + h * rows
                o = o_tiles[p]
                nc.sync.dma_start(
                    out_d[r0:r0 + rows, :].rearrange(
                        "(P q s) f -> q P s f", q=128, s=2),
                    o[:, h * MACRO_FREE:(h + 1) * MACRO_FREE].rearrange(
                        "p (P s f) -> p P s f", s=2, f=F_OUT))
                if sub_done[p] == 2:
                    o_tiles.pop(p)

            def drain_sub_one(engine):
                m2 = pending_subs.pop(0)
                sub(m2, engine)
                finish_sub(m2)

            n_iters = n_macros + 8
            for it in range(n_iters):
                # loads: 3 pairs ahead
                if it % 2 == 0:
                    pl = it // 2 + 3
                    if pl < n_pairs:
                        load(pl)

                m = it        # mm1 index
                if m < n_macros:
                    mm1(m)
                    silu0(m)
                if 0 <= m - 1 < n_macros:
                    h1t = mm2(m - 1)
                    silu1(m - 1, h1t)
                if 0 <= m - 2 < n_macros:
                    mm3(m - 2)
                    parkcopy(m - 2)
                    if pending_reduce2:
                        emit_reduce2()
                    gi, off = macro_group[m - 2]
                    if off == groups[gi] - 1:
                        exp_reduce(gi)
                        # big groups defer ln 3 iters (their 2x2.2us reduce
                        # halves must clear DVE first); small tail groups
                        # defer 1 so their subtracts/stores drain in-loop
                        defer = 3 if groups[gi] > 2 else 1
                        pending_ln.append((gi, it + defer))

                # deferred group ln: emit 3 iterations after its exp/reduce
                # so the DVE reduce is never on ACT's critical path
                if pending_ln and it >= pending_ln[0][1]:
                    gi2, _ = pending_ln.pop(0)
                    ln_group(gi2)
                    st = g_start[gi2]
                    pending_subs.extend(range(st, st + groups[gi2]))

                # tail unit: the <=128-row remainder rides the last xt tile
                # (feature-major columns past the last full macro); its whole
                # softmax runs out of PSUM on ACT/DVE and stores row-major
                if tail_rows and m == n_macros:
                    t = tail_rows
                    xoff = (n_macros % 2) * MACRO_ROWS
                    xtl = xt_tiles[n_pairs - 1]
                    h0tt = h0p.tile([128, MACRO_ROWS], _DT.float32,
                                    name="h0tt", tag="h0t")
                    nc.tensor.matmul(h0tt[:, 0:t], lhsT=w0,
                                     rhs=xtl[:, xoff:xoff + t],
                                     start=True, stop=True)
                    h0bt = h0s.tile([128, MACRO_ROWS], _DT.bfloat16, tag="h0b")
                    nc.scalar.activation(h0bt[:, 0:t], h0tt[:, 0:t], AF.Silu,
                                         bias=b0)
                    h1tt = h1p.tile([128, MACRO_ROWS], _DT.float32, tag="h1t")
                    nc.tensor.matmul(h1tt[:, 0:t], lhsT=w1, rhs=h0bt[:, 0:t],
                                     start=True, stop=True)
                    h1bt = h1s.tile([128, MACRO_ROWS], _DT.bfloat16, tag="h1b")
                    nc.scalar.activation(h1bt[:, 0:t], h1tt[:, 0:t], AF.Silu,
                                         bias=b1)
                    h2tt = h2p.tile([128, MACRO_FREE], _DT.float32, tag="h2t")
                    n_mm = 2 if with_b2 else 1
                    nc.tensor.matmul(h2tt[0:t, 0:F_OUT], lhsT=h1bt[:, 0:t],
                                     rhs=w2, start=True, stop=(n_mm == 1))
                    if with_b2:
                        nc.tensor.matmul(h2tt[0:t, 0:F_OUT],
                                         lhsT=ones1[:, 0:t], rhs=b2,
                                         start=False, stop=True)
                    et = epool.tile([128, GROUP_MACROS * MACRO_FREE],
                                    _DT.bfloat16, tag="e")
                    nc.scalar.activation(et[0:t, 0:F_OUT], h2tt[0:t, 0:F_OUT],
                                         AF.Exp)
                    St = spool.tile([128, GROUP_MACROS * MACRO_BLKS],
                                    _DT.float32, tag="s")
                    nc.vector.tensor_reduce(St[0:t, 0:1], et[0:t, 0:F_OUT],
                                            axis=mybir.AxisListType.X,
                                            op=mybir.AluOpType.add)
                    LZt = lzpool.tile([128, GROUP_MACROS * MACRO_BLKS],
                                      _DT.float32, tag="lz")
                    nc.scalar.activation(LZt[0:t, 0:1], St[0:t, 0:1], AF.Ln)
                    ot = opool.tile([128, PAIR_ROWS // 128 * F_OUT],
                                    _DT.float32, name="ot", tag="o")
                    nc.vector.tensor_tensor(
                        out=ot[0:t, 0:F_OUT], in0=h2tt[0:t, 0:F_OUT],
                        in1=LZt[0:t, 0:1].broadcast_to([t, F_OUT]),
                        op=mybir.AluOpType.subtract)
                    r0t = n_macros * MACRO_ROWS
                    nc.sync.dma_start(out_d[r0t:r0t + t, :], ot[0:t, 0:F_OUT])

                # pace subtracts: keep up with production (1/iter steady,
                # 2 when backlogged). Final groups run on DVE (Pool idles
                # in the tail and DVE's op is faster there).
                n_drain = 2 if len(pending_subs) > 2 else 1
                for _ in range(min(n_drain, len(pending_subs))):
                    drain_sub_one(sub_engine(pending_subs[0]))

            # ---- tail: flush remaining lns / subs / stores
            while pending_reduce2:
                emit_reduce2()
            while pending_ln:
                ln_group(pending_ln.pop(0)[0])
            done_macros = set()
            for p, cnt in sub_done.items():
                for i in range(cnt):
                    done_macros.add(p * 2 + i)
            rest = [m for m in range(n_macros) if m not in done_macros]
            # preserve order; all lns are emitted by now
            for m in rest:
                sub(m, sub_engine(m))
                finish_sub(m)

    return nc


_BUILD_CACHE = {}


def _get_module(nc_rows: int, with_b2: bool):
    key = (nc_rows, with_b2)
    if key not in _BUILD_CACHE:
        _BUILD_CACHE[key] = _build(nc_rows, with_b2)
    return _BUILD_CACHE[key]


def _prepare(x, W0, b0, W1, b1, W2, b2):
    """Shard + stage host-side inputs; returns (nc, in_maps, per, n)."""
    x = np.asarray(x)
    n = x.shape[0]
    per = _per_rows(n)
    total = per * N_CORES

    bf = ml_dtypes.bfloat16
    xp = np.zeros((total, F_IN), dtype=bf)
    xp[:n] = np.asarray(x, dtype=np.float32).astype(bf)

    w0b = np.ascontiguousarray(np.asarray(W0, dtype=np.float32)).astype(bf)
    w1b = np.ascontiguousarray(np.asarray(W1, dtype=np.float32)).astype(bf)
    w2b = np.ascontiguousarray(np.asarray(W2, dtype=np.float32)).astype(bf)
    b0f = np.asarray(b0, dtype=np.float32).reshape(F_HID, 1)
    b1f = np.asarray(b1, dtype=np.float32).reshape(F_HID, 1)
    b2f = np.asarray(b2, dtype=np.float32).reshape(1, F_OUT)
    with_b2 = bool(np.any(b2f))
    b2b = b2f.astype(bf)
    cpack = np.zeros((336, F_IN), dtype=bf)
    cpack[0, :] = b0f.reshape(-1).astype(bf)
    cpack[1, :] = b1f.reshape(-1).astype(bf)
    cpack[16:144, :] = w0b.T
    cpack[144:272, :] = w1b.T
    cpack[272:336, :] = w2b.T
    cpack = np.ascontiguousarray(cpack)

    nc = _get_module(per, with_b2)

    in_maps = []
    for i in range(N_CORES):
        in_maps.append({
            "x": xp[i * per:(i + 1) * per],
            "cpack": cpack, "b2": b2b,
        })
    return nc, in_maps, per, n


def _per_rows(n):
    """Per-core row count: ceil(n/8) rounded up to 16, bumped to the next
    full macro when the remainder exceeds the tail unit's 128-row limit."""
    per = ((math.ceil(n / N_CORES) + 15) // 16) * 16
    t = per % MACRO_ROWS
    if t and not 16 <= t <= 128:
        per = (per // MACRO_ROWS + 1) * MACRO_ROWS
    return per


def kernel(x, edge_index=None, W0=None, b0=None, W1=None, b1=None, W2=None,
           b2=None, **_unused):
    nc, in_maps, per, n = _prepare(x, W0, b0, W1, b1, W2, b2)
    res = run_bass_kernel_spmd(nc, in_maps, list(range(N_CORES)))
    out = np.concatenate([res.results[i]["out"] for i in range(N_CORES)],
                         axis=0)
    return np.ascontiguousarray(out[:n])
